# revision 1
# baseline (speedup 1.0000x reference)
"""SSIM(3x3 avg-pool) + L1 loss kernel for Trainium2, 8 NeuronCores.

loss = 0.85 * mean(clip((1 - ssim_map)/2, 0, 1)) + 0.15 * mean(|pred - target|)

Full inputs pred/target: (16, 1, 1024, 1024) f32. Data-parallel: 2 images per
core; each core returns per-partition partial sums [128, 2] (col 0 = sum of
the clipped ssim loss map, col 1 = sum |pred-target|); the host combines and
applies the means / alpha-beta weights.

Math (per image pair, variance identities halve the pooled field count):
  u = p + t, v = p - t
  box(x) = 3x3 zero-padded box sum / 9 (separable)
  X = box(p), Y = box(t), G = box(u^2), Hh = box(v^2)
  n1*n2 = (2XY + C1) * ((G-Hh)/2 - 2XY + C2)
  d1*d2 = (X^2+Y^2 + C1) * ((G+Hh)/2 - (X^2+Y^2) + C2)
  ssim_map = n1*n2/(d1*d2);  contrib = clip(0.5 - 0.5*ssim_map, 0, 1)
  l1 from |v|.

Layout: the host packs each image pair row-wise as [0 p0 0|0 p1 0|0 t0 0|0 t1 0]
(zero-padded sections of width W+2), so every pre-pool stage (horizontal 3-tap
via shifted adds, u/v, squares, |v|) is ONE wide instruction across both
images and both tensors. The vertical 3-tap runs on the TensorEngine as a
banded [128 -> <=126] float32r matmul per 512-col chunk into a single 8-bank
PSUM tile; row halos come from overlapped 128-row stripes, image edges from
per-block banded matrices. Post-pool math uses custom fused DVE ops
(x^2+y^2, the (a+c0)(b*c1-a+c2) rational terms, and a fused
clip-and-accumulate), one reciprocal_approx_fast for the division.
"""

import sys

import numpy as np

sys.path.insert(0, "/opt/trn_rl_repo")

ALPHA = 0.85
BETA = 0.15
C1 = 0.01 ** 2
C2 = 0.03 ** 2

N_CORES = 8
IMG_H = 1024
IMG_W = 1024
N_IMG_PER_CORE = 2
BLK = 126          # output rows per vertical-matmul block
MAXW_PSUM = 512    # fp32 columns per PSUM bank

MM_F32R = True     # float32r matmuls: 4x PE throughput, ~1e-6 rel error
MM_BF16 = False    # bf16 h3 boxes bias the clipped-mean by ~4% - keep f32r

# --- custom fused DVE ops (registered into concourse.dve_ops at build) ---- #
_OP_SQSUM = None       # out = in0^2 + in1^2
_OP_SSIM_RAT = None    # out = (in0 + s0) * (in1*s1 - in0 + imm2)
_OP_SSIM_FINAL = None  # out = (s0 - clamp(in0*in1, s1, s0))*imm2; accum += out
_CUSTOM_OPS_OK = False


def _register_custom_ops():
    global _OP_SQSUM, _OP_SSIM_RAT, _OP_SSIM_FINAL, _CUSTOM_OPS_OK
    if _CUSTOM_OPS_OK:
        return
    from operator import add

    import concourse.dve_ops as dv
    from concourse.dve_spec import (
        C0, C1 as SC1, C2 as SC2, Spec, Src0, Src1, Zero, lower, maxx, minn, sq,
    )
    from concourse.dve_uop import DveOpSpec

    def _sqsum_ref(in0, in1, c0, c1, c2):
        return in0.astype(np.float32) ** 2 + in1.astype(np.float32) ** 2

    def _rat_ref(in0, in1, c0, c1, c2):
        a = in0.astype(np.float32)
        return (a + c0) * (in1.astype(np.float32) * c1 - a + c2)

    def _final_ref(in0, in1, c0, c1, c2):
        z = in0.astype(np.float32) * in1.astype(np.float32)
        b = ((c0 - np.clip(z, c1, c0)) * c2).astype(np.float32)
        return b, b.reshape(b.shape[0], -1).sum(axis=-1, keepdims=True)

    defs = [
        ("SSIM_SQSUM_ANT", Spec(body=sq(Src0) + sq(Src1), reference=_sqsum_ref)),
        ("SSIM_RAT_ANT", Spec(
            body=(Src0 + C0) * (Src1 * SC1 - Src0 + SC2), reference=_rat_ref)),
        ("SSIM_FINAL_ANT", Spec(
            body=(C0 - maxx(minn(Src0 * Src1, C0), SC1)) * SC2,
            accum=add, accum_init=Zero, reference=_final_ref)),
    ]
    made = {}
    for name, spec in defs:
        if name not in dv._SUB_OPCODE_FOR_NAME:
            stub = dv.DveOp(name, spec, subdim=False, uops_sha={})
            dv.OPS.append(stub)
            dv._SUB_OPCODE_FOR_NAME[name] = (
                dv._CUSTOM_DVE_ROW_BASE + len(dv.OPS) - 1
            )
            dv.CUSTOM_DVE_SPECS[name] = spec
        opcode = dv._SUB_OPCODE_FOR_NAME[name]
        shas = {}
        for ver in ("v3", "v4"):
            res = DveOpSpec(
                name=name, opcode=opcode, uops=lower(spec, ver=ver),
                rd1_en=dv.has_src1(spec),
            )
            shas[ver] = res.sha(ver)
        op = dv.DveOp(name, spec, subdim=False, uops_sha=shas)
        idx = next(i for i, o in enumerate(dv.OPS) if o.name == name)
        dv.OPS[idx] = op
        dv.CUSTOM_DVE_SPECS[name] = spec
        made[name] = op
    _OP_SQSUM = made["SSIM_SQSUM_ANT"]
    _OP_SSIM_RAT = made["SSIM_RAT_ANT"]
    _OP_SSIM_FINAL = made["SSIM_FINAL_ANT"]
    _CUSTOM_OPS_OK = True


def _blocks(H):
    """Vertical block decomposition: list of (r0, n_out, rs, nr)."""
    out = []
    b = 0
    while b * BLK < H:
        r0 = b * BLK
        n_out = min(BLK, H - r0)
        rs = max(r0 - 1, 0)
        re = min(r0 + n_out, H - 1)
        out.append((r0, n_out, rs, re - rs + 1))
        b += 1
    return out


def make_bmats(H):
    """Banded vertical-sum matrices, padded into [nblk,128,BLK]. Entries are
    1/9 (normalized boxes) or 1.0 when MM_BF16 (bf16-exact; the /9 moves into
    81-scaled ssim constants)."""
    blocks = _blocks(H)
    bm = np.zeros((len(blocks), 128, BLK), dtype=np.float32)
    ninth = np.float32(1.0) if MM_BF16 else np.float32(1.0) / np.float32(9.0)
    for i, (r0, n_out, rs, nr) in enumerate(blocks):
        for k in range(nr):
            for j in range(n_out):
                if abs((rs + k) - (r0 + j)) <= 1:
                    bm[i, k, j] = ninth
    return bm


def build_program(n_img, H, W, io_internal=False):
    """Build the per-core program for n_img (even) HxW images.

    DRAM input "ptin": [ (n_img/2)*H, 4*(W+2) ] with row layout
    [0 p0 0 | 0 p1 0 | 0 t0 0 | 0 t1 0] per image pair.
    io_internal makes ptin Internal DRAM (timing-only builds).
    """
    import concourse.bacc as bacc
    import concourse.tile as tile
    from concourse import mybir

    assert n_img % 2 == 0
    f32 = mybir.dt.float32
    Alu = mybir.AluOpType
    Act = mybir.ActivationFunctionType

    blocks = _blocks(H)
    nblk = len(blocks)
    S = W + 2                       # one padded section
    S4 = 4 * S                      # packed row width
    npairs = n_img // 2
    n_chunks = (W + MAXW_PSUM - 1) // MAXW_PSUM
    W2 = 2 * W                      # field-pair width (img0|img1)

    _register_custom_ops()
    nc = bacc.Bacc("TRN2", target_bir_lowering=False, debug=False)

    io_kind = "Internal" if io_internal else "ExternalInput"
    ptin_d = nc.dram_tensor("ptin", [npairs * H, S4], f32, kind=io_kind).ap()
    bm_d = nc.dram_tensor("bmats", [nblk, 128, BLK], f32, kind="ExternalInput").ap()
    acc_d = nc.dram_tensor("acc_out", [128, 2], f32, kind="ExternalOutput").ap()

    with tile.TileContext(nc) as tc:
        with (
            tc.tile_pool(name="consts", bufs=1) as cpool,
            tc.tile_pool(name="io", bufs=2) as iopool,
            tc.tile_pool(name="hsum", bufs=2) as hpool,
            tc.tile_pool(name="post", bufs=1) as ppool,
            tc.tile_pool(name="psum", bufs=1, space="PSUM") as psumpool,
        ):
            acc = cpool.tile([128, 2], f32, tag="acc")
            nc.vector.memset(acc[:, :], 0.0)
            if io_internal:
                fill = cpool.tile([128, S4], f32, tag="fill")
                nc.vector.memset(fill[:, :], 0.625)
                rows_total = npairs * H
                for r in range(0, rows_total, 128):
                    nrr = min(128, rows_total - r)
                    nc.sync.dma_start(out=ptin_d[r:r + nrr, :], in_=fill[0:nrr, :])

            mm_dt = (mybir.dt.bfloat16 if MM_BF16
                     else (mybir.dt.float32r if MM_F32R else f32))
            mm_n = MAXW_PSUM
            n_chunks_mm = (W + mm_n - 1) // mm_n
            cC1 = float(C1) * (81.0 if MM_BF16 else 1.0)
            cC2 = float(C2) * (81.0 if MM_BF16 else 1.0)
            bmats = []
            for i, (r0, n_out, rs, nr) in enumerate(blocks):
                braw = cpool.tile([128, BLK], f32, tag=f"bmraw{i}", name="braw")
                nc.sync.dma_start(out=braw[0:nr, 0:n_out], in_=bm_d[i, 0:nr, 0:n_out])
                if MM_F32R or MM_BF16:
                    bt = cpool.tile([128, BLK], mm_dt, tag=f"bmat{i}", name="bt")
                    nc.vector.tensor_copy(bt[0:nr, 0:n_out], braw[0:nr, 0:n_out])
                else:
                    bt = braw
                bmats.append(bt)

            for pair in range(npairs):
                base = pair * H
                for bi, (r0, n_out, rs, nr) in enumerate(blocks):
                    # rows [0:k_l1] of consecutive stripes tile H exactly once
                    if bi + 1 < len(blocks):
                        k_l1 = blocks[bi + 1][2] - rs
                    else:
                        k_l1 = nr

                    pt = iopool.tile([128, S4], f32, tag="pt")
                    nc.sync.dma_start(
                        out=pt[0:nr, :], in_=ptin_d[base + rs: base + rs + nr, :])

                    rows = slice(0, nr)
                    # horizontal 3-tap for p0,p1,t0,t1 in two ops
                    # (junk at section tails is never read)
                    g = hpool.tile([128, S4 - 1], f32, tag="g")
                    nc.vector.tensor_add(
                        g[rows, :], pt[rows, 0:S4 - 1], pt[rows, 1:S4])
                    h3pt = hpool.tile([128, S4 - 2], mm_dt, tag="h3pt")
                    nc.vector.tensor_add(
                        h3pt[rows, :], g[rows, 0:S4 - 2], pt[rows, 2:S4])

                    # in place: t-half <- v = p - t ; p-half <- u = 2p - v
                    nc.gpsimd.tensor_sub(
                        pt[rows, 2 * S:S4], pt[rows, 0:2 * S], pt[rows, 2 * S:S4])
                    nc.vector.scalar_tensor_tensor(
                        pt[rows, 0:2 * S], pt[rows, 0:2 * S], 2.0,
                        pt[rows, 2 * S:S4], op0=Alu.mult, op1=Alu.subtract)
                    # L1 partial: |v| in place over the disjoint-cover rows
                    l1part = ppool.tile([128, 1], f32, tag="l1part")
                    nc.scalar.activation(
                        pt[0:k_l1, 2 * S:S4], pt[0:k_l1, 2 * S:S4], Act.Abs,
                        accum_out=l1part[0:k_l1, :])
                    # squares in place: [u0 u1 v0 v1] -> [u0^2 u1^2 v0^2 v1^2]
                    nc.scalar.activation(pt[rows, :], pt[rows, :], Act.Square)

                    g2 = hpool.tile([128, S4 - 1], f32, tag="g", name="g2")
                    nc.vector.tensor_add(
                        g2[rows, :], pt[rows, 0:S4 - 1], pt[rows, 1:S4])
                    h3uv = hpool.tile([128, S4 - 2], mm_dt, tag="h3uv")
                    nc.vector.tensor_add(
                        h3uv[rows, :], g2[rows, 0:S4 - 2], pt[rows, 2:S4])

                    bmat = bmats[bi]
                    ro = slice(0, n_out)
                    pw = slice(0, W2)

                    def mm_group(h3, ps):
                        # fields [f0_img0|f0_img1|f1_img0|f1_img1] -> PSUM
                        for s in range(4):
                            for ci in range(n_chunks_mm):
                                c0 = ci * mm_n
                                cw = min(mm_n, W - c0)
                                nc.tensor.matmul(
                                    ps[0:n_out, s * W + c0: s * W + c0 + cw],
                                    lhsT=bmat[0:nr, 0:n_out],
                                    rhs=h3[0:nr, s * S + c0: s * S + c0 + cw],
                                    start=True, stop=True)

                    # group 1: X|Y
                    ps = psumpool.tile([128, 4 * W], f32, tag="ps", name="ps")
                    mm_group(h3pt, ps)
                    Ysb = ppool.tile([128, W2], f32, tag="Ysb", name="Ysb")
                    nc.scalar.copy(Ysb[ro, :], ps[ro, W2:4 * W])
                    A2 = ppool.tile([128, W2], f32, tag="A2", name="A2")
                    nc.vector.scalar_tensor_tensor(
                        A2[ro, pw], ps[ro, 0:W2], 2.0, Ysb[ro, pw],
                        op0=Alu.mult, op1=Alu.mult)
                    V = ppool.tile([128, W2], f32, tag="V", name="V")
                    nc.vector._custom_dve(
                        _OP_SQSUM, out=V[ro, pw], in0=ps[ro, 0:W2], in1=Ysb[ro, pw])

                    # group 2: G|Hh (reuses the PSUM banks)
                    ps2 = psumpool.tile([128, 4 * W], f32, tag="ps", name="ps2")
                    mm_group(h3uv, ps2)
                    Hsb = ppool.tile([128, W2], f32, tag="Hsb", name="Hsb")
                    nc.scalar.copy(Hsb[ro, :], ps2[ro, W2:4 * W])
                    Dd = ppool.tile([128, W2], f32, tag="Dd", name="Dd")
                    nc.vector.tensor_sub(Dd[ro, pw], ps2[ro, 0:W2], Hsb[ro, pw])
                    M = ppool.tile([128, W2], f32, tag="M", name="M")
                    nc.vector.tensor_add(M[ro, pw], ps2[ro, 0:W2], Hsb[ro, pw])

                    # in-place: n1n2 -> A2's tile, d1d2 -> V, rcp -> M, fin -> Dd
                    n1n2 = A2
                    nc.vector._custom_dve(
                        _OP_SSIM_RAT, out=n1n2[ro, pw], in0=A2[ro, pw],
                        in1=Dd[ro, pw], s0=cC1, s1=0.5, imm2=cC2)
                    d1d2 = V
                    nc.vector._custom_dve(
                        _OP_SSIM_RAT, out=d1d2[ro, pw], in0=V[ro, pw],
                        in1=M[ro, pw], s0=cC1, s1=0.5, imm2=cC2)
                    rcp = M
                    nc.vector.reciprocal_approx_fast(rcp[ro, pw], d1d2[ro, pw])
                    fin = Dd
                    spart = ppool.tile([128, 1], f32, tag="spart")
                    nc.vector._custom_dve(
                        _OP_SSIM_FINAL, out=fin[ro, pw], in0=n1n2[ro, pw],
                        in1=rcp[ro, pw], s0=1.0, s1=-1.0, imm2=0.5,
                        accum_out=spart[ro, :])
                    nc.vector.tensor_add(
                        acc[0:n_out, 0:1], acc[0:n_out, 0:1], spart[ro, :])
                    nc.vector.tensor_add(
                        acc[0:k_l1, 1:2], acc[0:k_l1, 1:2], l1part[0:k_l1, :])

            nc.sync.dma_start(out=acc_d[:, :], in_=acc[:, :])

    nc.compile()
    return nc


_CACHE = {}


def _get_program(n_img, H, W):
    key = (n_img, H, W)
    if key not in _CACHE:
        _CACHE[key] = build_program(n_img, H, W)
    return _CACHE[key]


def _pack_inputs(pred, target):
    """pred/target [n_img, H, W] -> packed [npairs*H, 4*(W+2)]."""
    n_img, H, W = pred.shape
    assert n_img % 2 == 0
    npairs = n_img // 2
    S = W + 2
    out = np.zeros((npairs * H, 4 * S), dtype=np.float32)
    out[:, 1:W + 1] = pred[0::2].reshape(npairs * H, W)
    out[:, S + 1:S + W + 1] = pred[1::2].reshape(npairs * H, W)
    out[:, 2 * S + 1:2 * S + W + 1] = target[0::2].reshape(npairs * H, W)
    out[:, 3 * S + 1:3 * S + W + 1] = target[1::2].reshape(npairs * H, W)
    return out


LAST_RESULTS = None


def kernel(pred, target):
    from concourse.bass_utils import run_bass_kernel_spmd

    global LAST_RESULTS

    pred = np.asarray(pred, dtype=np.float32).reshape(16, IMG_H, IMG_W)
    target = np.asarray(target, dtype=np.float32).reshape(16, IMG_H, IMG_W)

    nc = _get_program(N_IMG_PER_CORE, IMG_H, IMG_W)
    bm = make_bmats(IMG_H)

    in_maps = []
    for c in range(N_CORES):
        sl = slice(c * N_IMG_PER_CORE, (c + 1) * N_IMG_PER_CORE)
        in_maps.append({
            "ptin": _pack_inputs(pred[sl], target[sl]),
            "bmats": bm,
        })

    res = run_bass_kernel_spmd(nc, in_maps, list(range(N_CORES)))
    LAST_RESULTS = res
    ssim_sum = 0.0
    l1_sum = 0.0
    for r in res.results:
        acc = r["acc_out"]
        ssim_sum += float(acc[:, 0].sum(dtype=np.float64))
        l1_sum += float(acc[:, 1].sum(dtype=np.float64))
    n = 16.0 * IMG_H * IMG_W
    loss = ALPHA * (ssim_sum / n) + BETA * (l1_sum / n)
    return np.float32(loss)



# revision 21
# speedup vs baseline: 4.6068x; 4.6068x over previous
"""SSIM(3x3 avg-pool) + L1 loss kernel for Trainium2, 8 NeuronCores.

loss = 0.85 * mean(clip((1 - ssim_map)/2, 0, 1)) + 0.15 * mean(|pred - target|)

Full inputs pred/target: (16, 1, 1024, 1024) f32. Data-parallel: 2 images per
core, processed as two sequential passes; each core returns per-partition
partial sums acc_out [128, 4] (cols: ssim0, l1_0, ssim1, l1_1); the host
combines and applies the means / alpha-beta weights.

This environment charges ~50 us of fixed overhead per *instruction*
(engine- and width-independent), so the kernel is built around a minimal
instruction count (~24 per image) of maximally wide ops:

  u = p + t, v = p - t (box3 is linear: with BU = boxsum(u), BV = boxsum(v),
  G = boxsum(u^2), H = boxsum(v^2):
    2*mu_x*mu_y    = (BU^2 - BV^2)/162     x^2+y^2-terms = (BU^2 + BV^2)/162
    2*sigma_xy + C2 = (G - H)/18 - 2XY + C2  (folded into the rational op)
    sx + sy + C2    = (G + H)/18 - (X^2+Y^2) + C2

Layout: the host packs each image as overlapping 126-row stripes with a
leading zero row, [128 part, 9 stripes, 2*(W+2)] with row content
[0 p_row 0 | 0 t_row 0], in bf16. Per image pass:
  - one DMA in; u/v/|v|/squares as whole-tile ops; horizontal 3-tap as two
    wide adds (second in-place); vertical 3-tap as 1 copy-DMA + 2
    accumulate-DMAs (gpsimd SWDGE accum_op=add) per field pair - pooling
    costs 6 DMA instructions instead of 144 matmuls.
  - rational/clip via custom fused DVE ops at full [126, 9, 1024] width.
Zero padding makes all junk regions (stripe tails, pad cols) contribute
exactly 0 to both accumulated sums, so no masking ops are needed.
"""

import sys

import numpy as np

sys.path.insert(0, "/opt/trn_rl_repo")

ALPHA = 0.85
BETA = 0.15
C1 = 0.01 ** 2
C2 = 0.03 ** 2

N_CORES = 8
IMG_H = 1024
IMG_W = 1024
N_IMG_PER_CORE = 2

NSTRIPE = 9          # ceil(1024 / 126) output stripes of 126 rows
SEC = IMG_W + 2      # one padded section [0 row 0]
ROWW = 2 * SEC       # packed row width [p-section | t-section]
PAD_ROWS = 126 * (NSTRIPE - 1) + 128  # padded image rows incl. top zero

# --- custom fused DVE ops ------------------------------------------------- #
_OP_SQS = None         # out = (Src0^2 + Src1^2) * c0
_OP_SQD = None         # out = (Src0^2 - Src1^2) * c0
_OP_SSIM_RAT = None    # out = (in0 + s0) * (in1*s1 - in0 + imm2)
_OP_SSIM_FINAL = None  # out = (s0 - clamp(in0*in1, s1, s0))*imm2; accum += out
_CUSTOM_OPS_OK = False


def _register_custom_ops():
    global _OP_SQS, _OP_SQD, _OP_SSIM_RAT, _OP_SSIM_FINAL, _CUSTOM_OPS_OK
    if _CUSTOM_OPS_OK:
        return
    from operator import add

    import concourse.dve_ops as dv
    from concourse.dve_spec import (
        C0, C1 as SC1, C2 as SC2, Spec, Src0, Src1, Zero, lower, maxx, minn,
        sq,
    )
    from concourse.dve_uop import DveOpSpec

    def _sqs_ref(in0, in1, c0, c1, c2):
        a = in0.astype(np.float32)
        b = in1.astype(np.float32)
        return (a * a + b * b) * c0

    def _sqd_ref(in0, in1, c0, c1, c2):
        a = in0.astype(np.float32)
        b = in1.astype(np.float32)
        return (a * a - b * b) * c0

    def _rat_ref(in0, in1, c0, c1, c2):
        a = in0.astype(np.float32)
        return (a + c0) * (in1.astype(np.float32) * c1 - a + c2)

    def _final_ref(in0, in1, c0, c1, c2):
        z = in0.astype(np.float32) * in1.astype(np.float32)
        b = ((c0 - np.clip(z, c1, c0)) * c2).astype(np.float32)
        return b, b.reshape(b.shape[0], -1).sum(axis=-1, keepdims=True)

    defs = [
        ("SSIM_SQS_ANT", Spec(
            body=(sq(Src0) + sq(Src1)) * C0, reference=_sqs_ref)),
        ("SSIM_SQD_ANT", Spec(
            body=(sq(Src0) - sq(Src1)) * C0, reference=_sqd_ref)),
        ("SSIM_RAT_ANT", Spec(
            body=(Src0 + C0) * (Src1 * SC1 - Src0 + SC2), reference=_rat_ref)),
        ("SSIM_FINAL_ANT", Spec(
            body=(C0 - maxx(minn(Src0 * Src1, C0), SC1)) * SC2,
            accum=add, accum_init=Zero, reference=_final_ref)),
    ]
    made = {}
    for name, spec in defs:
        if name not in dv._SUB_OPCODE_FOR_NAME:
            stub = dv.DveOp(name, spec, subdim=False, uops_sha={})
            dv.OPS.append(stub)
            dv._SUB_OPCODE_FOR_NAME[name] = (
                dv._CUSTOM_DVE_ROW_BASE + len(dv.OPS) - 1
            )
            dv.CUSTOM_DVE_SPECS[name] = spec
        opcode = dv._SUB_OPCODE_FOR_NAME[name]
        shas = {}
        for ver in ("v3", "v4"):
            res = DveOpSpec(
                name=name, opcode=opcode, uops=lower(spec, ver=ver),
                rd1_en=dv.has_src1(spec),
            )
            shas[ver] = res.sha(ver)
        op = dv.DveOp(name, spec, subdim=False, uops_sha=shas)
        idx = next(i for i, o in enumerate(dv.OPS) if o.name == name)
        dv.OPS[idx] = op
        dv.CUSTOM_DVE_SPECS[name] = spec
        made[name] = op
    _OP_SQS = made["SSIM_SQS_ANT"]
    _OP_SQD = made["SSIM_SQD_ANT"]
    _OP_SSIM_RAT = made["SSIM_RAT_ANT"]
    _OP_SSIM_FINAL = made["SSIM_FINAL_ANT"]
    _CUSTOM_OPS_OK = True


def build_program(n_img, H, W, io_internal=False, max_stage=99):
    """Per-core program for n_img HxW images (one pass per image).

    DRAM input "ptin": [n_img, 128, NSTRIPE, ROWW] bf16, host-packed
    overlapping stripes (see _pack_inputs). io_internal makes ptin Internal
    DRAM (timing-only builds).
    """
    import concourse.bacc as bacc
    import concourse.tile as tile
    from concourse import mybir

    assert H == IMG_H and W == IMG_W
    f32 = mybir.dt.float32
    bf16 = mybir.dt.bfloat16
    Alu = mybir.AluOpType
    Act = mybir.ActivationFunctionType

    S = SEC
    NS = NSTRIPE
    n = NS * ROWW                     # flattened free-dim length

    _register_custom_ops()
    nc = bacc.Bacc("TRN2", target_bir_lowering=False, debug=False)

    io_kind = "Internal" if io_internal else "ExternalInput"
    ptin_d = nc.dram_tensor(
        "ptin", [n_img, 128, NS, ROWW], bf16, kind=io_kind).ap()
    acc_d = nc.dram_tensor(
        "acc_out", [128, 4 * n_img], f32, kind="ExternalOutput").ap()

    with tile.TileContext(nc) as tc:
        with tc.tile_pool(name="main", bufs=1) as pool:
            acc = pool.tile([128, 4 * n_img], f32, tag="acc", name="acc")
            nc.vector.memset(acc[:, :], 0.0)
            if io_internal:
                fill = pool.tile([128, n], bf16, tag="A", name="fill")
                nc.vector.memset(fill[:, :], 0.125)
                for i in range(n_img):
                    nc.sync.dma_start(
                        out=ptin_d[i].rearrange("p s c -> p (s c)"),
                        in_=fill[:, :])

            for i in range(n_img):
                # A <- packed [0 p 0 | 0 t 0] stripes, bf16
                A = pool.tile([128, NS, ROWW], bf16, tag="A", name="A")
                nc.sync.dma_start(out=A[:, :, :], in_=ptin_d[i])
                Af = A.rearrange("p s c -> p (s c)")

                if max_stage < 2:
                    continue
                psec = A[:, :, 0:S]          # p-half (per stripe)
                tsec = A[:, :, S:ROWW]       # t-half
                # v = p - t (in place in t-half); u = 2p - v (p-half)
                nc.gpsimd.tensor_sub(tsec, psec, tsec)
                nc.vector.scalar_tensor_tensor(
                    psec, psec, 2.0, tsec, op0=Alu.mult, op1=Alu.subtract)

                if max_stage < 3:
                    continue
                # L1 partials: |v| over all rows (stripe-overlap rows counted
                # twice -> tracked in a dup column the host subtracts;
                # compute engines must start at partition 0/32/64/96).
                B = pool.tile([128, NS, ROWW], bf16, tag="B", name="B")
                nc.scalar.activation(
                    B[:, :, S:ROWW], A[:, :, S:ROWW], Act.Abs,
                    accum_out=acc[:, 4 * i + 2:4 * i + 3])
                nc.scalar.activation(
                    B[0:2, 1:NS, S:ROWW], A[0:2, 1:NS, S:ROWW], Act.Abs,
                    accum_out=acc[0:2, 4 * i + 3:4 * i + 4])
                # squares: B <- [u^2 | v^2]
                nc.scalar.activation(B[:, :, :], A[:, :, :], Act.Square)

                if max_stage < 4:
                    continue
                # horizontal 3-tap, in place:  x[j] <- x[j] + x[j+1] + x[j+2]
                g = pool.tile([128, n], bf16, tag="g", name="g")
                Bf = B.rearrange("p s c -> p (s c)")
                nc.vector.tensor_add(g[:, 0:n - 1], Af[:, 0:n - 1],
                                     Af[:, 1:n])
                nc.vector.tensor_add(Af[:, 0:n - 2], g[:, 0:n - 2],
                                     Af[:, 2:n])
                nc.vector.tensor_add(g[:, 0:n - 1], Bf[:, 0:n - 1],
                                     Bf[:, 1:n])
                nc.vector.tensor_add(Bf[:, 0:n - 2], g[:, 0:n - 2],
                                     Bf[:, 2:n])

                if max_stage < 5:
                    continue
                # vertical 3-tap: partition-shifted HWDGE copies + aligned
                # DVE adds (compute engines cannot read shifted partitions;
                # DMA-accum silently breaks above ~4KB/partition):
                # C[p] = A[p] + A[p+1] + A[p+2]  ->  [BU | BV]
                ro = slice(0, 126)
                Cp = pool.tile([128, NS, ROWW], bf16, tag="S1", name="Cp")
                S2 = pool.tile([128, NS, ROWW], bf16, tag="S2", name="S2")
                nc.sync.dma_start(out=Cp[0:126], in_=A[1:127])
                nc.sync.dma_start(out=S2[0:126], in_=A[2:128])
                nc.gpsimd.tensor_add(Cp[ro], A[ro], Cp[ro])
                nc.gpsimd.tensor_add(Cp[ro], S2[ro], Cp[ro])
                # D = [G | H] (third shift lands in A's region, now free)
                Dp = pool.tile([128, NS, ROWW], bf16, tag="S2", name="Dp")
                shB = pool.tile([128, NS, ROWW], bf16, tag="A", name="shB")
                nc.sync.dma_start(out=Dp[0:126], in_=B[1:127])
                nc.sync.dma_start(out=shB[0:126], in_=B[2:128])
                nc.gpsimd.tensor_add(Dp[ro], B[ro], Dp[ro])
                nc.gpsimd.tensor_add(Dp[ro], shB[ro], Dp[ro])

                # valid pooled views: [126, NS, W]
                BU = Cp[ro, :, 0:W]
                BV = Cp[ro, :, S:S + W]
                Gg = Dp[ro, :, 0:W]
                Hh = Dp[ro, :, S:S + W]

                if max_stage < 6:
                    continue
                s162 = 1.0 / 162.0
                nw = NS * W
                # V = (BU^2+BV^2)/162 -> tag B ; A2 = (BU^2-BV^2)/162 -> tag A
                # (flat [126, NS*W] tiles: RAT/FINAL imm2 needs 1-free-dim in1)
                Vt = pool.tile([126, nw], f32, tag="B", name="Vt")
                nc.vector._custom_dve(
                    _OP_SQS, out=Vt[:, :], in0=BU, in1=BV, s0=s162)
                A2 = pool.tile([126, nw], f32, tag="A", name="A2")
                nc.vector._custom_dve(
                    _OP_SQD, out=A2[:, :], in0=BU, in1=BV, s0=s162)
                # M = G + H -> tag g ; Dd = G - H -> tag S1
                M = pool.tile([126, nw], f32, tag="g", name="M")
                nc.vector.tensor_add(M[:, :], Gg, Hh)
                Dd = pool.tile([126, nw], f32, tag="S1", name="Dd")
                nc.vector.tensor_sub(Dd[:, :], Gg, Hh)

                if max_stage < 7:
                    continue
                s18 = 1.0 / 18.0
                # n1n2 in place over A2; d1d2 in place over Vt
                nc.vector._custom_dve(
                    _OP_SSIM_RAT, out=A2[:, :], in0=A2[:, :],
                    in1=Dd[:, :], s0=float(C1), s1=s18, imm2=float(C2))
                nc.vector._custom_dve(
                    _OP_SSIM_RAT, out=Vt[:, :], in0=Vt[:, :],
                    in1=M[:, :], s0=float(C1), s1=s18, imm2=float(C2))
                nc.vector.reciprocal_approx_fast(Vt[:, :], Vt[:, :])
                # accumulate only valid pooled rows (126s+p < H): stripes
                # 0..7 in full, stripe 8 rows 0..15 -- junk rows would
                # otherwise leak rcp-approx noise (~1e-3 each) into the sum
                nmain = (NS - 1) * W
                ntail = H - 126 * (NS - 1)
                nc.vector._custom_dve(
                    _OP_SSIM_FINAL, out=A2[:, 0:nmain],
                    in0=A2[:, 0:nmain], in1=Vt[:, 0:nmain],
                    s0=1.0, s1=-1.0, imm2=0.5,
                    accum_out=acc[0:126, 4 * i:4 * i + 1])
                nc.vector._custom_dve(
                    _OP_SSIM_FINAL, out=A2[0:ntail, nmain:nw],
                    in0=A2[0:ntail, nmain:nw], in1=Vt[0:ntail, nmain:nw],
                    s0=1.0, s1=-1.0, imm2=0.5,
                    accum_out=acc[0:ntail, 4 * i + 1:4 * i + 2])

            nc.sync.dma_start(out=acc_d[:, :], in_=acc[:, :])

    nc.compile()
    return nc


_CACHE = {}


def _get_program(n_img, H, W):
    key = (n_img, H, W)
    if key not in _CACHE:
        _CACHE[key] = build_program(n_img, H, W)
    return _CACHE[key]


def _pack_inputs(pred, target):
    """pred/target [n_img, H, W] f32 -> packed [n_img, 128, NS, ROWW] bf16.

    Padded image rows: row 0 zero, rows 1..H the image, zeros below.
    stripe s / partition p holds padded row 126*s + p.
    Row content: [0 p_row 0 | 0 t_row 0].
    """
    import ml_dtypes

    n_img, H, W = pred.shape
    bf = ml_dtypes.bfloat16
    padded = np.zeros((n_img, PAD_ROWS, ROWW), dtype=bf)
    padded[:, 1:H + 1, 1:W + 1] = pred.astype(bf)
    padded[:, 1:H + 1, SEC + 1:SEC + W + 1] = target.astype(bf)
    st = np.lib.stride_tricks.as_strided(
        padded,
        shape=(n_img, NSTRIPE, 128, ROWW),
        strides=(padded.strides[0], 126 * padded.strides[1],
                 padded.strides[1], padded.strides[2]),
    )
    return np.ascontiguousarray(st.transpose(0, 2, 1, 3))


LAST_RESULTS = None


def kernel(pred, target):
    from concourse.bass_utils import run_bass_kernel_spmd

    global LAST_RESULTS

    pred = np.asarray(pred, dtype=np.float32).reshape(16, IMG_H, IMG_W)
    target = np.asarray(target, dtype=np.float32).reshape(16, IMG_H, IMG_W)

    nc = _get_program(N_IMG_PER_CORE, IMG_H, IMG_W)

    in_maps = []
    for c in range(N_CORES):
        sl = slice(c * N_IMG_PER_CORE, (c + 1) * N_IMG_PER_CORE)
        in_maps.append({"ptin": _pack_inputs(pred[sl], target[sl])})

    res = run_bass_kernel_spmd(nc, in_maps, list(range(N_CORES)))
    LAST_RESULTS = res
    ssim_sum = 0.0
    l1_sum = 0.0
    for r in res.results:
        acc = r["acc_out"]
        ssim_sum += float(acc[:, 0::4].sum(dtype=np.float64))
        ssim_sum += float(acc[:, 1::4].sum(dtype=np.float64))
        l1_sum += float(acc[:, 2::4].sum(dtype=np.float64))
        l1_sum -= float(acc[:, 3::4].sum(dtype=np.float64))
    n = 16.0 * IMG_H * IMG_W
    loss = ALPHA * (ssim_sum / n) + BETA * (l1_sum / n)
    return np.float32(loss)
